# revision 4
# baseline (speedup 1.0000x reference)
"""Trainium2 Bass kernel for nn_ComputationGraphTableParse.

Math (iterations=1, hh=0 => D/B2_Wh dead, only B_W[:500] used):
  uu  = tanh(vv_aug @ A_W_aug)                       # A_b folded
  bb  = tanh(sum_s gather_s(uu) @ W_s + B_b)         # gather via dma_gather
  oo  = tanh(bb @ B2_Wo + B2_bo)
  per head: hl = oo@W1a + b1, hr = oo@W1b
  out[i,j,:] = relu(hl[i]+hr[j]) @ W2 + b2           # (N,N,2) x 3 heads

Sharding: SPMD over 8 cores; core c receives word-permuted inputs
(roll by 96c) and computes pairwise rows i'=0..95 (global i = 96c+i').
Pre-stage replicated per core (cheap). No collectives.

Pairwise packing: units (head,i) are assigned to (round r, shift u,
col-group g); each matmul uses a shifted-W2 (100,32) stationary at
tile_position (0,32g) so 64 units/round accumulate into a dense
(128,768) PSUM block -> full-partition evacuation.
"""
import os
import sys
import time
import numpy as np
import ml_dtypes

sys.path.insert(0, "/opt/trn_rl_repo")

N, H, NCORES, IPC = 768, 100, 8, 96
UNITS = 3 * IPC            # 288 (head, i) units per core
NIDX = 5 * N               # 3840 gathered rows
ROUNDS = 5                 # ceil(288/64)

LAST_EXEC_NS = None
LAST_WALL_S = None
_CACHE = {}


def _build_program():
    import concourse.bass as bass
    import concourse.tile as tile
    from concourse import mybir

    dt = mybir.dt
    F = mybir.ActivationFunctionType
    A = mybir.AluOpType
    f32, bf16, i16 = dt.float32, dt.bfloat16, dt.int16
    f32r = dt.float32r

    nc = bass.Bass("TRN2", target_bir_lowering=False, debug=False,
                   num_devices=NCORES)

    # ---- external inputs (per-core data) ----
    vvT_d = nc.dram_tensor("vvT", [309, N], f32, kind="ExternalInput")
    AW_d = nc.dram_tensor("AW", [309, H], f32, kind="ExternalInput")
    idxs_d = nc.dram_tensor("idxs", [16, NIDX // 16], i16, kind="ExternalInput")
    Ws_d = nc.dram_tensor("Ws", [H, 500], bf16, kind="ExternalInput")
    B2Wo_d = nc.dram_tensor("B2Wo", [H, H], f32, kind="ExternalInput")
    W1a_d = nc.dram_tensor("W1a", [H, 300], f32, kind="ExternalInput")
    W1b_d = nc.dram_tensor("W1b", [H, 300], f32, kind="ExternalInput")
    W2s_d = nc.dram_tensor("W2s", [H, 48 * 32], f32, kind="ExternalInput")
    bias_d = nc.dram_tensor("bias", [H, 5], f32, kind="ExternalInput")
    b2e_d = nc.dram_tensor("b2e", [128, ROUNDS], f32, kind="ExternalInput")

    # ---- internal DRAM + output ----
    uu_d = nc.dram_tensor("uu_scratch", [776, 128], bf16)
    out_d = nc.dram_tensor("out", [ROUNDS, 128, N], f32, kind="ExternalOutput")

    CH = [(0, 512), (512, 256)]  # j-chunks: both >=256 so f32r is 1 cyc/row

    with tile.TileContext(nc) as tc:
        cpool = tc.alloc_tile_pool(name="consts", bufs=1)
        # persistent SBUF tensors (outer pool, live whole kernel)
        big = tc.alloc_tile_pool(name="big", bufs=1)

        # load weights
        vvT_sb = []
        AW_sb = []
        for d0, dsz in ((0, 128), (128, 128), (256, 53)):
            v = cpool.tile([dsz, N], f32, name=f"vvT{d0}")
            nc.sync.dma_start(out=v[:], in_=vvT_d.ap()[d0:d0 + dsz, :])
            vvT_sb.append(v)
            a = cpool.tile([dsz, H], f32, name=f"AW{d0}")
            nc.sync.dma_start(out=a[:], in_=AW_d.ap()[d0:d0 + dsz, :])
            AW_sb.append(a)
        Ws_sb = cpool.tile([H, 500], bf16)
        nc.sync.dma_start(out=Ws_sb[:], in_=Ws_d.ap()[:])
        B2Wo_sb = cpool.tile([H, H], f32)
        nc.sync.dma_start(out=B2Wo_sb[:], in_=B2Wo_d.ap()[:])
        W1a_sb = cpool.tile([H, 300], f32)
        nc.sync.dma_start(out=W1a_sb[:], in_=W1a_d.ap()[:])
        W1b_sb = cpool.tile([H, 300], f32)
        nc.sync.dma_start(out=W1b_sb[:], in_=W1b_d.ap()[:])
        W2s_sb = cpool.tile([H, 48 * 32], f32)
        nc.sync.dma_start(out=W2s_sb[:], in_=W2s_d.ap()[:])
        bias_sb = cpool.tile([H, 5], f32)
        nc.sync.dma_start(out=bias_sb[:], in_=bias_d.ap()[:])
        b2e_sb = cpool.tile([128, ROUNDS], f32)
        nc.sync.dma_start(out=b2e_sb[:], in_=b2e_d.ap()[:])
        idxs_sb = cpool.tile([128, NIDX // 16], i16)
        nc.vector.memset(idxs_sb[:], 0)
        nc.sync.dma_start(out=idxs_sb[0:16, :], in_=idxs_d.ap()[:])

        G_sb = big.tile([128, NIDX], bf16)           # gathered uu^T
        hlT = [big.tile([H, N], f32, name=f"hlT{h}") for h in range(3)]
        hrT = [big.tile([H, N], f32, name=f"hrT{h}") for h in range(3)]

        # ---------------- pre-stage ----------------
        with tc.tile_pool(name="pre_sb", bufs=2) as psb, \
             tc.tile_pool(name="pre_ps", bufs=2, space="PSUM") as pps:
            # uu rows -> DRAM (bf16, padded to 128 cols)
            for ti in range(6):
                ps = pps.tile([128, H], f32, tag="uu_ps")
                for d in range(3):
                    d0 = (0, 128, 256)[d]
                    nc.tensor.matmul(
                        ps[:], lhsT=vvT_sb[d][:, ti * 128:(ti + 1) * 128],
                        rhs=AW_sb[d][:], start=(d == 0), stop=(d == 2))
                ut = psb.tile([128, 128], bf16, tag="uu_sb")
                nc.vector.memset(ut[:], 0)
                nc.scalar.activation(ut[:, 0:H], ps[:], F.Tanh)
                nc.sync.dma_start(out=uu_d.ap()[ti * 128:(ti + 1) * 128, :],
                                  in_=ut[:])
            zt = psb.tile([8, 128], bf16, tag="zrow")
            nc.vector.memset(zt[:], 0)
            nc.sync.dma_start(out=uu_d.ap()[768:776, :], in_=zt[:])

            # gather: G[p, s*768+n] = uu[idx[n,s], p]
            from concourse import library_config
            nc.gpsimd.load_library(library_config.mlp)
            nc.gpsimd.dma_gather(
                out_ap=G_sb[:].rearrange("p (o j) -> p o j", o=1),
                in_ap=uu_d.ap(),
                idxs_ap=idxs_sb[:],
                num_idxs=NIDX, num_idxs_reg=NIDX,
                elem_size=128, transpose=True)

            # bb^T = tanh(sum_s Ws^T @ G_s + B_b)
            bbT = psb.tile([H, N], f32, tag="bbT")
            for ci, (joff, jsz) in enumerate(CH):
                ps = pps.tile([H, jsz], f32, tag=f"pre{ci}")
                for s in range(5):
                    nc.tensor.matmul(
                        ps[:], lhsT=Ws_sb[:, s * H:(s + 1) * H],
                        rhs=G_sb[0:H, s * N + joff:s * N + joff + jsz],
                        start=(s == 0), stop=(s == 4))
                nc.scalar.activation(bbT[:, joff:joff + jsz], ps[:], F.Tanh,
                                     bias=bias_sb[:, 0:1])
            # oo^T = tanh(B2Wo^T @ bb^T + bo)
            ooT = psb.tile([H, N], f32, tag="ooT")
            for ci, (joff, jsz) in enumerate(CH):
                ps = pps.tile([H, jsz], f32, tag=f"pre{ci}")
                nc.tensor.matmul(ps[:], lhsT=B2Wo_sb[:].bitcast(f32r),
                                 rhs=bbT[:, joff:joff + jsz].bitcast(f32r),
                                 start=True, stop=True)
                nc.scalar.activation(ooT[:, joff:joff + jsz], ps[:], F.Tanh,
                                     bias=bias_sb[:, 1:2])
            # hl^T (+b1) / hr^T per head
            for h in range(3):
                for ci, (joff, jsz) in enumerate(CH):
                    ps = pps.tile([H, jsz], f32, tag=f"pre{ci}")
                    nc.tensor.matmul(
                        ps[:], lhsT=W1a_sb[:, h * H:(h + 1) * H].bitcast(f32r),
                        rhs=ooT[:, joff:joff + jsz].bitcast(f32r),
                        start=True, stop=True)
                    nc.scalar.add(hlT[h][:, joff:joff + jsz], ps[:],
                                  bias_sb[:, 2 + h:3 + h])
                for ci, (joff, jsz) in enumerate(CH):
                    ps = pps.tile([H, jsz], f32, tag=f"pre{ci}")
                    nc.tensor.matmul(
                        ps[:], lhsT=W1b_sb[:, h * H:(h + 1) * H].bitcast(f32r),
                        rhs=ooT[:, joff:joff + jsz].bitcast(f32r),
                        start=True, stop=True)
                    nc.scalar.copy(hrT[h][:, joff:joff + jsz], ps[:])

        # ---------------- pairwise stage ----------------
        with tc.tile_pool(name="T_pool", bufs=8) as tpool, \
             tc.tile_pool(name="ev_pool", bufs=2) as evpool, \
             tc.tile_pool(name="pair_ps", bufs=2, space="PSUM") as ppool:
            for r in range(ROUNDS):
                pA = ppool.tile([128, 512], f32, tag="pA")
                pB = ppool.tile([128, 256], f32, tag="pB")
                for k in range(64):
                    unit = r * 64 + k
                    if unit >= UNITS:
                        break
                    g, u = k % 4, k // 4
                    head, i_loc = unit // IPC, unit % IPC
                    stop = (u == 15) or (unit + 4 >= UNITS)
                    T = tpool.tile([H, N], f32, tag="T")
                    if unit % 4 == 3:
                        nc.scalar.activation(T[:], hrT[head][:], F.Relu,
                                             bias=hlT[head][:, i_loc:i_loc + 1])
                    else:
                        nc.vector.tensor_scalar(
                            T[:], hrT[head][:],
                            scalar1=hlT[head][:, i_loc:i_loc + 1],
                            scalar2=0.0, op0=A.add, op1=A.max)
                    woff = (head * 16 + u) * 32
                    for ci, (joff, jsz) in enumerate(CH):
                        ps = (pA, pB)[ci]
                        nc.tensor.matmul(
                            ps[32 * g:32 * g + 32, :],
                            lhsT=W2s_sb[:, woff:woff + 32].bitcast(f32r),
                            rhs=T[:, joff:joff + jsz].bitcast(f32r),
                            start=(u == 0), stop=stop,
                            skip_group_check=True,
                            tile_position=(0, 32 * g))
                ev = evpool.tile([128, N], f32, tag="ev")
                nc.scalar.add(ev[:, 0:512], pA[:], b2e_sb[:, r:r + 1])
                nc.scalar.add(ev[:, 512:768], pB[:], b2e_sb[:, r:r + 1])
                nc.sync.dma_start(out=out_d.ap()[r], in_=ev[:])

        big.release()
        cpool.release()
    return nc


def _host_prep(inputs):
    """Build per-core input maps + shared weight tensors."""
    f32 = np.float32
    vv = np.asarray(inputs['vv'], f32)
    indices = np.asarray(inputs['indices']).astype(np.int64)
    nf = np.asarray(inputs['indices_not_found']).astype(bool)
    A_W = np.asarray(inputs['A_W'], f32)
    A_b = np.asarray(inputs['A_b'], f32)
    AW_aug = np.concatenate([A_W, A_b[None, :]], 0)          # (309,100)
    B_W0 = np.asarray(inputs['B_W'], f32)[:500]              # (500,100)
    heads = ['rows', 'cols', 'cells']

    Ws_host = np.zeros((H, 500), ml_dtypes.bfloat16)
    for s in range(5):
        Ws_host[:, s * H:(s + 1) * H] = B_W0[s * H:(s + 1) * H]
    W1a = np.concatenate([np.asarray(inputs[h + '_W1'], f32)[:H]
                          for h in heads], 1)                # (100,300)
    W1b = np.concatenate([np.asarray(inputs[h + '_W1'], f32)[H:]
                          for h in heads], 1)
    W2s = np.zeros((H, 48 * 32), f32)
    for h in range(3):
        W2 = np.asarray(inputs[heads[h] + '_W2'], f32)       # (100,2)
        for u in range(16):
            W2s[:, (h * 16 + u) * 32 + 2 * u:(h * 16 + u) * 32 + 2 * u + 2] = W2
    bias = np.zeros((H, 5), f32)
    bias[:, 0] = np.asarray(inputs['B_b'], f32)
    bias[:, 1] = np.asarray(inputs['B2_bo'], f32)
    for h in range(3):
        bias[:, 2 + h] = np.asarray(inputs[heads[h] + '_b1'], f32)
    b2 = [np.asarray(inputs[h + '_b2'], f32) for h in heads]
    b2e = np.zeros((128, ROUNDS), f32)
    for r in range(ROUNDS):
        for p in range(128):
            unit = r * 64 + ((p % 32) // 2) * 4 + p // 32
            if unit < UNITS:
                b2e[p, r] = b2[unit // IPC][p % 2]

    idx0 = np.where(nf[:, None], 768, indices)               # (768,5)
    shared = dict(AW=AW_aug, Ws=Ws_host, B2Wo=np.asarray(inputs['B2_Wo'], f32),
                  W1a=W1a, W1b=W1b, W2s=W2s, bias=bias, b2e=b2e)
    in_maps = []
    for c in range(NCORES):
        r_amt = IPC * c
        perm = (np.arange(N) + r_amt) % N
        vvT = np.concatenate([vv[perm], np.ones((N, 1), f32)], 1).T.copy()
        ip = idx0[perm]
        idxp = np.where(ip == 768, 768, (ip - r_amt) % N)
        idx_flat = idxp.T.reshape(-1)                        # j = s*768+n
        idxs = idx_flat.reshape(NIDX // 16, 16).T.astype(np.int16).copy()
        in_maps.append(dict(vvT=np.ascontiguousarray(vvT), idxs=idxs, **shared))
    return in_maps


def _host_post(vals):
    """vals: list of 8 arrays (ROUNDS,128,768) -> tuple of 3 (N,N,2)."""
    units = np.arange(UNITS)
    gg, uu_, rr = units % 4, (units // 4) % 16, units // 64
    out = np.zeros((3, N, N, 2), np.float32)
    for c in range(NCORES):
        val = np.asarray(vals[c], np.float32)
        r_amt = IPC * c
        for ch in range(2):
            pp = 32 * gg + 2 * uu_ + ch
            Aarr = val[rr, pp, :].reshape(3, IPC, N)
            out[:, r_amt:r_amt + IPC, :, ch] = np.roll(Aarr, r_amt, axis=2)
    return (out[0], out[1], out[2])


def kernel(**inputs):
    global LAST_EXEC_NS, LAST_WALL_S
    if 'nc' not in _CACHE:
        _CACHE['nc'] = _build_program()
    nc = _CACHE['nc']
    in_maps = _host_prep(inputs)

    if os.environ.get("BASS_SIM") == "1":
        from concourse.bass_interp import CoreSim
        vals = []
        ncsim = int(os.environ.get("BASS_SIM_CORES", "1"))
        for c in range(ncsim):
            sim = CoreSim(nc, core_id=0)
            for k, v in in_maps[c].items():
                sim.tensor(k)[:] = v
            sim.simulate()
            vals.append(sim.tensor("out").copy())
        vals += [np.zeros((ROUNDS, 128, N), np.float32)] * (NCORES - ncsim)
        return _host_post(vals)

    from concourse.bass_utils import run_bass_kernel_spmd
    trace = os.environ.get("BASS_TRACE_RUN") == "1"
    t0 = time.monotonic()
    res = run_bass_kernel_spmd(nc, in_maps, list(range(NCORES)), trace=trace)
    LAST_WALL_S = time.monotonic() - t0
    LAST_EXEC_NS = res.exec_time_ns
    vals = [res.results[c]["out"] for c in range(NCORES)]
    return _host_post(vals)


# revision 7
# speedup vs baseline: 1.4069x; 1.4069x over previous
"""Trainium2 Bass kernel for nn_ComputationGraphTableParse.

Math (iterations=1, hh=0 => D/B2_Wh dead, only B_W[:500] used):
  uu  = tanh(vv_aug @ A_W_aug)                       # A_b folded
  bb  = tanh(sum_s gather_s(uu) @ W_s + B_b)         # gather via dma_gather
  oo  = tanh(bb @ B2_Wo + B2_bo)
  per head: hl = oo@W1a + b1, hr = oo@W1b
  out[i,j,:] = relu(hl[i]+hr[j]) @ W2 + b2           # (N,N,2) x 3 heads

Sharding: SPMD over 8 cores; core c receives word-permuted inputs
(roll by 96c) and computes pairwise rows i'=0..95 (global i = 96c+i').
Pre-stage replicated per core (cheap). No collectives.

Pairwise packing: units (head,i) are assigned to (round r, shift u,
col-group g); each matmul uses a shifted-W2 (100,32) stationary at
tile_position (0,32g) so 64 units/round accumulate into a dense
(128,768) PSUM block -> full-partition evacuation.
"""
import os
import sys
import time
import numpy as np
import ml_dtypes

sys.path.insert(0, "/opt/trn_rl_repo")

N, H, NCORES, IPC = 768, 100, 8, 96
UNITS = 3 * IPC            # 288 (head, i) units per core
NIDX = 5 * N               # 3840 gathered rows
ROUNDS = 5                 # ceil(288/64)

LAST_EXEC_NS = None
LAST_WALL_S = None
_CACHE = {}


def _build_program():
    import concourse.bass as bass
    import concourse.tile as tile
    from concourse import mybir

    dt = mybir.dt
    F = mybir.ActivationFunctionType
    A = mybir.AluOpType
    f32, bf16, i16 = dt.float32, dt.bfloat16, dt.int16
    f32r = dt.float32r

    nc = bass.Bass("TRN2", target_bir_lowering=False, debug=False,
                   num_devices=NCORES)

    # ---- external inputs (per-core data) ----
    vvT_d = nc.dram_tensor("vvT", [309, N], f32, kind="ExternalInput")
    AW_d = nc.dram_tensor("AW", [309, H], f32, kind="ExternalInput")
    idxs_d = nc.dram_tensor("idxs", [16, NIDX // 16], i16, kind="ExternalInput")
    Ws_d = nc.dram_tensor("Ws", [H, 500], bf16, kind="ExternalInput")
    B2Wo_d = nc.dram_tensor("B2Wo", [H, H], f32r, kind="ExternalInput")
    W1a_d = nc.dram_tensor("W1a", [H, 300], f32r, kind="ExternalInput")
    W1b_d = nc.dram_tensor("W1b", [H, 300], f32r, kind="ExternalInput")
    W2s_d = nc.dram_tensor("W2s", [H, 48 * 32], f32r, kind="ExternalInput")
    bias_d = nc.dram_tensor("bias", [H, 5], f32, kind="ExternalInput")
    b2e_d = nc.dram_tensor("b2e", [128, ROUNDS], f32, kind="ExternalInput")

    # ---- internal DRAM + output ----
    uu_d = nc.dram_tensor("uu_scratch", [776, 128], bf16)
    out_d = nc.dram_tensor("out", [ROUNDS, 128, N], f32, kind="ExternalOutput")

    CH = [(0, 512), (512, 256)]  # j-chunks: both >=256 so f32r is 1 cyc/row

    with tile.TileContext(nc) as tc:
        cpool = tc.alloc_tile_pool(name="consts", bufs=1)
        # persistent SBUF tensors (outer pool, live whole kernel)
        big = tc.alloc_tile_pool(name="big", bufs=1)

        # load weights
        vvT_sb = []
        AW_sb = []
        for d0, dsz in ((0, 128), (128, 128), (256, 53)):
            v = cpool.tile([dsz, N], f32, name=f"vvT{d0}")
            nc.sync.dma_start(out=v[:], in_=vvT_d.ap()[d0:d0 + dsz, :])
            vvT_sb.append(v)
            a = cpool.tile([dsz, H], f32, name=f"AW{d0}")
            nc.sync.dma_start(out=a[:], in_=AW_d.ap()[d0:d0 + dsz, :])
            AW_sb.append(a)
        Ws_sb = cpool.tile([H, 500], bf16)
        nc.sync.dma_start(out=Ws_sb[:], in_=Ws_d.ap()[:])
        B2Wo_sb = cpool.tile([H, H], f32r)
        nc.sync.dma_start(out=B2Wo_sb[:], in_=B2Wo_d.ap()[:])
        W1a_sb = cpool.tile([H, 300], f32r)
        nc.sync.dma_start(out=W1a_sb[:], in_=W1a_d.ap()[:])
        W1b_sb = cpool.tile([H, 300], f32r)
        nc.sync.dma_start(out=W1b_sb[:], in_=W1b_d.ap()[:])
        W2s_sb = cpool.tile([H, 48 * 32], f32r)
        nc.sync.dma_start(out=W2s_sb[:], in_=W2s_d.ap()[:])
        bias_sb = cpool.tile([H, 5], f32)
        nc.sync.dma_start(out=bias_sb[:], in_=bias_d.ap()[:])
        b2e_sb = cpool.tile([128, ROUNDS], f32)
        nc.sync.dma_start(out=b2e_sb[:], in_=b2e_d.ap()[:])
        idxs_sb = cpool.tile([128, NIDX // 16], i16)
        nc.vector.memset(idxs_sb[:], 0)
        nc.sync.dma_start(out=idxs_sb[0:16, :], in_=idxs_d.ap()[:])

        G_sb = big.tile([128, NIDX], bf16)           # gathered uu^T
        hlT = [big.tile([H, N], f32, name=f"hlT{h}") for h in range(3)]
        hrT = [big.tile([H, N], f32, name=f"hrT{h}") for h in range(3)]

        # ---------------- pre-stage ----------------
        with tc.tile_pool(name="pre_sb", bufs=2) as psb, \
             tc.tile_pool(name="pre_ps", bufs=2, space="PSUM") as pps:
            # uu rows -> DRAM (bf16, padded to 128 cols)
            for ti in range(6):
                ps = pps.tile([128, H], f32, tag="uu_ps")
                for d in range(3):
                    d0 = (0, 128, 256)[d]
                    nc.tensor.matmul(
                        ps[:], lhsT=vvT_sb[d][:, ti * 128:(ti + 1) * 128],
                        rhs=AW_sb[d][:], start=(d == 0), stop=(d == 2))
                ut = psb.tile([128, 128], bf16, tag="uu_sb")
                nc.vector.memset(ut[:], 0)
                nc.scalar.activation(ut[:, 0:H], ps[:], F.Tanh)
                nc.sync.dma_start(out=uu_d.ap()[ti * 128:(ti + 1) * 128, :],
                                  in_=ut[:])
            zt = psb.tile([8, 128], bf16, tag="zrow")
            nc.vector.memset(zt[:], 0)
            nc.sync.dma_start(out=uu_d.ap()[768:776, :], in_=zt[:])

            # gather: G[p, s*768+n] = uu[idx[n,s], p]
            from concourse import library_config
            nc.gpsimd.load_library(library_config.mlp)
            nc.gpsimd.dma_gather(
                out_ap=G_sb[:].rearrange("p (o j) -> p o j", o=1),
                in_ap=uu_d.ap(),
                idxs_ap=idxs_sb[:],
                num_idxs=NIDX, num_idxs_reg=NIDX,
                elem_size=128, transpose=True)

            # bb^T = tanh(sum_s Ws^T @ G_s + B_b)
            bbT = psb.tile([H, N], f32r, tag="bbT")
            for ci, (joff, jsz) in enumerate(CH):
                ps = pps.tile([H, jsz], f32, tag=f"pre{ci}")
                for s in range(5):
                    nc.tensor.matmul(
                        ps[:], lhsT=Ws_sb[:, s * H:(s + 1) * H],
                        rhs=G_sb[0:H, s * N + joff:s * N + joff + jsz],
                        start=(s == 0), stop=(s == 4))
                nc.scalar.activation(bbT[:, joff:joff + jsz], ps[:], F.Tanh,
                                     bias=bias_sb[:, 0:1])
            # oo^T = tanh(B2Wo^T @ bb^T + bo)
            ooT = psb.tile([H, N], f32r, tag="ooT")
            for ci, (joff, jsz) in enumerate(CH):
                ps = pps.tile([H, jsz], f32, tag=f"pre{ci}")
                nc.tensor.matmul(ps[:], lhsT=B2Wo_sb[:],
                                 rhs=bbT[:, joff:joff + jsz],
                                 start=True, stop=True)
                nc.scalar.activation(ooT[:, joff:joff + jsz], ps[:], F.Tanh,
                                     bias=bias_sb[:, 1:2])
            # hl^T (+b1) / hr^T per head
            for h in range(3):
                for ci, (joff, jsz) in enumerate(CH):
                    ps = pps.tile([H, jsz], f32, tag=f"pre{ci}")
                    nc.tensor.matmul(
                        ps[:], lhsT=W1a_sb[:, h * H:(h + 1) * H],
                        rhs=ooT[:, joff:joff + jsz],
                        start=True, stop=True)
                    nc.scalar.add(hlT[h][:, joff:joff + jsz], ps[:],
                                  bias_sb[:, 2 + h:3 + h])
                for ci, (joff, jsz) in enumerate(CH):
                    ps = pps.tile([H, jsz], f32, tag=f"pre{ci}")
                    nc.tensor.matmul(
                        ps[:], lhsT=W1b_sb[:, h * H:(h + 1) * H],
                        rhs=ooT[:, joff:joff + jsz],
                        start=True, stop=True)
                    nc.scalar.copy(hrT[h][:, joff:joff + jsz], ps[:])

        # ---------------- pairwise stage ----------------
        with tc.tile_pool(name="T_pool", bufs=8) as tpool, \
             tc.tile_pool(name="ev_pool", bufs=2) as evpool, \
             tc.tile_pool(name="pair_ps", bufs=2, space="PSUM") as ppool:
            for r in range(ROUNDS):
                pA = ppool.tile([128, 512], f32, tag="pA")
                pB = ppool.tile([128, 256], f32, tag="pB")
                for k in range(64):
                    unit = r * 64 + k
                    if unit >= UNITS:
                        break
                    g, u = k % 4, k // 4
                    head, i_loc = unit // IPC, unit % IPC
                    stop = (u == 15) or (unit + 4 >= UNITS)
                    T = tpool.tile([H, N], f32r, tag="T")
                    if unit % 4 == 3:
                        nc.scalar.activation(T[:], hrT[head][:], F.Relu,
                                             bias=hlT[head][:, i_loc:i_loc + 1])
                    else:
                        nc.vector.tensor_scalar(
                            T[:], hrT[head][:],
                            scalar1=hlT[head][:, i_loc:i_loc + 1],
                            scalar2=0.0, op0=A.add, op1=A.max)
                    woff = (head * 16 + u) * 32
                    for ci, (joff, jsz) in enumerate(CH):
                        ps = (pA, pB)[ci]
                        nc.tensor.matmul(
                            ps[32 * g:32 * g + 32, :],
                            lhsT=W2s_sb[:, woff:woff + 32],
                            rhs=T[:, joff:joff + jsz],
                            start=(u == 0), stop=stop,
                            skip_group_check=True,
                            tile_position=(0, 32 * g))
                ev = evpool.tile([128, N], f32, tag="ev")
                nc.scalar.add(ev[:, 0:512], pA[:], b2e_sb[:, r:r + 1])
                nc.scalar.add(ev[:, 512:768], pB[:], b2e_sb[:, r:r + 1])
                nc.sync.dma_start(out=out_d.ap()[r], in_=ev[:])

        big.release()
        cpool.release()
    return nc


def _host_prep(inputs):
    """Build per-core input maps + shared weight tensors."""
    f32 = np.float32
    vv = np.asarray(inputs['vv'], f32)
    indices = np.asarray(inputs['indices']).astype(np.int64)
    nf = np.asarray(inputs['indices_not_found']).astype(bool)
    A_W = np.asarray(inputs['A_W'], f32)
    A_b = np.asarray(inputs['A_b'], f32)
    AW_aug = np.concatenate([A_W, A_b[None, :]], 0)          # (309,100)
    B_W0 = np.asarray(inputs['B_W'], f32)[:500]              # (500,100)
    heads = ['rows', 'cols', 'cells']

    Ws_host = np.zeros((H, 500), ml_dtypes.bfloat16)
    for s in range(5):
        Ws_host[:, s * H:(s + 1) * H] = B_W0[s * H:(s + 1) * H]
    W1a = np.concatenate([np.asarray(inputs[h + '_W1'], f32)[:H]
                          for h in heads], 1)                # (100,300)
    W1b = np.concatenate([np.asarray(inputs[h + '_W1'], f32)[H:]
                          for h in heads], 1)
    W2s = np.zeros((H, 48 * 32), f32)
    for h in range(3):
        W2 = np.asarray(inputs[heads[h] + '_W2'], f32)       # (100,2)
        for u in range(16):
            W2s[:, (h * 16 + u) * 32 + 2 * u:(h * 16 + u) * 32 + 2 * u + 2] = W2
    bias = np.zeros((H, 5), f32)
    bias[:, 0] = np.asarray(inputs['B_b'], f32)
    bias[:, 1] = np.asarray(inputs['B2_bo'], f32)
    for h in range(3):
        bias[:, 2 + h] = np.asarray(inputs[heads[h] + '_b1'], f32)
    b2 = [np.asarray(inputs[h + '_b2'], f32) for h in heads]
    b2e = np.zeros((128, ROUNDS), f32)
    for r in range(ROUNDS):
        for p in range(128):
            unit = r * 64 + ((p % 32) // 2) * 4 + p // 32
            if unit < UNITS:
                b2e[p, r] = b2[unit // IPC][p % 2]

    idx0 = np.where(nf[:, None], 768, indices)               # (768,5)
    shared = dict(AW=AW_aug, Ws=Ws_host, B2Wo=np.asarray(inputs['B2_Wo'], f32),
                  W1a=W1a, W1b=W1b, W2s=W2s, bias=bias, b2e=b2e)
    in_maps = []
    for c in range(NCORES):
        r_amt = IPC * c
        perm = (np.arange(N) + r_amt) % N
        vvT = np.concatenate([vv[perm], np.ones((N, 1), f32)], 1).T.copy()
        ip = idx0[perm]
        idxp = np.where(ip == 768, 768, (ip - r_amt) % N)
        idx_flat = idxp.T.reshape(-1)                        # j = s*768+n
        idxs = idx_flat.reshape(NIDX // 16, 16).T.astype(np.int16).copy()
        in_maps.append(dict(vvT=np.ascontiguousarray(vvT), idxs=idxs, **shared))
    return in_maps


def _host_post(vals):
    """vals: list of 8 arrays (ROUNDS,128,768) -> tuple of 3 (N,N,2)."""
    units = np.arange(UNITS)
    gg, uu_, rr = units % 4, (units // 4) % 16, units // 64
    out = np.zeros((3, N, N, 2), np.float32)
    for c in range(NCORES):
        val = np.asarray(vals[c], np.float32)
        r_amt = IPC * c
        for ch in range(2):
            pp = 32 * gg + 2 * uu_ + ch
            Aarr = val[rr, pp, :].reshape(3, IPC, N)
            out[:, r_amt:r_amt + IPC, :, ch] = np.roll(Aarr, r_amt, axis=2)
    return (out[0], out[1], out[2])


def _install_profile_hook():
    """Inject antenv.axon_hooks with a ctypes NTFF hook (missing on this
    image) so run_bass_kernel_spmd(trace=True) can capture profiles."""
    import types
    import ctypes
    import contextlib
    try:
        from antenv.axon_hooks import get_axon_ntff_profile_hook  # noqa
        return
    except ImportError:
        pass
    try:
        lib = ctypes.CDLL('/opt/axon/libaxon_pjrt.so')
        if not hasattr(lib, 'axon_start_nrt_profile'):
            return
    except OSError:
        return
    lib.axon_start_nrt_profile.argtypes = [ctypes.POINTER(ctypes.c_int64),
                                           ctypes.c_size_t]
    lib.axon_start_nrt_profile.restype = ctypes.c_int64
    lib.axon_stop_nrt_profile.argtypes = [ctypes.c_char_p]
    lib.axon_stop_nrt_profile.restype = ctypes.c_int64

    @contextlib.contextmanager
    def _hook(output_dir, device_ids):
        import jax
        jax.devices()
        if device_ids:
            ids = (ctypes.c_int64 * len(device_ids))(*device_ids)
            rc = lib.axon_start_nrt_profile(ids, len(device_ids))
        else:
            rc = lib.axon_start_nrt_profile(None, 0)
        if rc != 0:
            raise RuntimeError(f'axon_start_nrt_profile rc={rc}')
        try:
            yield
        finally:
            n = lib.axon_stop_nrt_profile(str(output_dir).encode())
            print(f'profile: {n} ntff file(s) -> {output_dir}')

    mod = types.ModuleType('antenv.axon_hooks')
    mod.get_axon_ntff_profile_hook = lambda: _hook
    mod.set_axon_ntff_profile_hook = lambda h: None
    import antenv
    antenv.axon_hooks = mod
    sys.modules['antenv.axon_hooks'] = mod
    # artifact upload needs bucket creds we don't have; keep it local
    from concourse import bass_utils as _bu
    _bu.upload_artifacts = lambda tmpdir: tmpdir


def kernel(**inputs):
    global LAST_EXEC_NS, LAST_WALL_S
    if 'nc' not in _CACHE:
        _CACHE['nc'] = _build_program()
    nc = _CACHE['nc']
    in_maps = _host_prep(inputs)

    if os.environ.get("BASS_SIM") == "1":
        from concourse.bass_interp import CoreSim
        vals = []
        ncsim = int(os.environ.get("BASS_SIM_CORES", "1"))
        for c in range(ncsim):
            sim = CoreSim(nc, core_id=0)
            for k, v in in_maps[c].items():
                sim.tensor(k)[:] = v
            sim.simulate()
            vals.append(sim.tensor("out").copy())
        vals += [np.zeros((ROUNDS, 128, N), np.float32)] * (NCORES - ncsim)
        return _host_post(vals)

    from concourse.bass_utils import run_bass_kernel_spmd
    trace = os.environ.get("BASS_TRACE_RUN") == "1"
    if trace:
        _install_profile_hook()
    t0 = time.monotonic()
    res = run_bass_kernel_spmd(nc, in_maps, list(range(NCORES)), trace=trace)
    LAST_WALL_S = time.monotonic() - t0
    LAST_EXEC_NS = res.exec_time_ns
    vals = [res.results[c]["out"] for c in range(NCORES)]
    return _host_post(vals)
